# revision 17
# baseline (speedup 1.0000x reference)
"""Trainium2 Bass kernel for location-sensitive attention.

alpha = softmax(w_score . tanh(enc @ W_enc + b_enc + h @ W_dec + conv(prev_alpha) @ W_c2s)) * mask

Sharding: data-parallel over batch B=32 across 8 cores (4 batches/core).
All weights replicated. Full inputs in, full output out.

v3 design (vs v2): the rank-14 correction term (dec_e + b_enc + conv term)
is folded INTO the enc operand on the host via the pseudo-inverse of W_enc:
    (enc + small @ pinv(W_enc)) @ W_enc = enc @ W_enc + small
so the device runs a PURE 8-matmul-per-chunk stream -- the 9th (Mext)
matmul per chunk, the Hankel/hext DMA, and the Mext setup group are all
gone.  Min-norm spreads the correction across all 1024 contraction dims,
so bf16 error grows only ~1.2x (measured 1.03e-2 vs 2e-2 gate).

Device layout:
  - enchat is packed octant-major: encT[b, p, o*2048 + ki*256 + to] =
    enchat[b, o*256+to, ki*128+p], t zero-padded to 2048.  Each t-chunk
    (128 cols) and each DMA slice is contiguous; padded t rows produce
    e=0 which the mask kills in the softmax tail.
  - Per chunk: 8 accumulating matmuls (lhsT = enc chunk [128k, 128t],
    rhs = W chunk [128k, 512a]) -> tanh (Scalar ACT) -> fused
    tensor_tensor_reduce (x w_score, sum over a) -> e_t column.
  - DMA is issue-cost-aware (each dma_start costs ~0.7us of sequencer
    time): batch 0 lands as 8 octant slices + 4 wsb strips spread over
    the sync/scalar/gpsimd rings in consumption order; batches 1-3
    prefetch as 2 big strips each.  ~25 junk warmup matmuls run during
    the initial DMA wait to take the PE HAM throttle to 8/8 before the
    real stream starts.
  - softmax tail as v2 (exp, fused mask-mul+reduce, ones-matmul partition
    total, reciprocal, scale), deferred into the next batch's MM stream;
    the final batch's tail is split so only column 15's exp is exposed.
"""

import os
import sys
import numpy as np

for _p in ("/opt/trn_rl_repo", "/root/.axon_site/_ro/trn_rl_repo"):
    if os.path.isdir(_p) and _p not in sys.path:
        sys.path.append(_p)

import ml_dtypes
import concourse.bass as bass
import concourse.bacc as bacc
import concourse.mybir as mybir
from concourse import bass_isa
from concourse import tile

B, T, ENC2, DEC, ATTN = 32, 2000, 1024, 512, 512
NK, KW, PAD = 10, 100, 50
NCORES = 8
BPC = B // NCORES  # batches per core

F32 = mybir.dt.float32
BF16 = mybir.dt.bfloat16
AF = mybir.ActivationFunctionType
ALU = mybir.AluOpType
BF = ml_dtypes.bfloat16

KCH = ENC2 // 128   # 8 k-chunks of the 1024 contraction
NC_T = 16           # t-chunks of 128 (chunk 15 is 80 real + 48 zero pad)
TPAD = NC_T * 128   # 2048
NO = 8              # octants of 256 t
OCT = KCH * 256     # 2048 cols per octant in the packed layout
NWARM = int(os.environ.get("K_NWARM", "40"))   # junk warmup matmuls
USE_TTR = os.environ.get("K_TTR", "1") == "1"  # fused mul+reduce on DVE
SPLIT_FINAL = os.environ.get("K_SPLITFIN", "1") == "1"


def build_nc():
    nc = bacc.Bacc(None, target_bir_lowering=False)

    encT = nc.declare_dram_parameter("encT", [BPC, 128, NO * OCT], BF16, isOutput=False)
    wsb = nc.declare_dram_parameter("wsb", [128, KCH * ATTN], BF16, isOutput=False)
    wbc = nc.declare_dram_parameter("wbc", [128, ATTN], BF16, isOutput=False)
    maskt = nc.declare_dram_parameter("maskt", [BPC, 128, NC_T], F32, isOutput=False)
    out = nc.declare_dram_parameter("out", [BPC, 128, NC_T], F32, isOutput=True)

    with tile.TileContext(nc) as tc:
        with (
            tc.tile_pool(name="const", bufs=1) as cpool,
            tc.tile_pool(name="enc", bufs=3) as enc_pool,
            tc.tile_pool(name="th", bufs=3) as th_pool,
            tc.tile_pool(name="scr", bufs=2) as scr_pool,
            tc.tile_pool(name="tail", bufs=2) as tail_pool,
            tc.tile_pool(name="mask", bufs=4) as mask_pool,
            tc.tile_pool(name="pacc", bufs=5, space="PSUM") as pacc_pool,
            tc.tile_pool(name="pset", bufs=1, space="PSUM") as pset_pool,
        ):
            # ---- constants: memsets on the Vector engine so the gpsimd ring
            # can start issuing DMAs immediately ----
            onesb = cpool.tile([128, 128], BF16)
            nc.vector.memset(onesb[:, :], 1.0)
            ones128 = cpool.tile([128, 128], F32)
            nc.vector.memset(ones128[:, :], 1.0)

            wsb_sb = cpool.tile([128, KCH * ATTN], BF16)
            wbc_sb = cpool.tile([128, ATTN], BF16)

            # ---- startup DMAs spread over the three DGE rings.  Rings
            # pipeline only a few outstanding transfers and all share the 16
            # DMA queues, so queue-ENTRY order is delivery order.  The
            # chunk-0 critical set (wsb 1MB + octant 0) is the first 1-2
            # issues on every ring so it drains at full aggregate bandwidth;
            # later octants and the batch-1 prefetch queue up behind it. ----
            et0 = enc_pool.tile([128, NO * OCT], BF16, tag="encT")
            HOCT = OCT // 2

            def wsb_strip(eng, s):
                eng.dma_start(wsb_sb[:, s * 2 * ATTN : (s + 1) * 2 * ATTN],
                              wsb[:, s * 2 * ATTN : (s + 1) * 2 * ATTN])

            def oct_piece(eng, lo, hi):
                eng.dma_start(et0[:, lo:hi], encT[0, :, lo:hi])

            wsb_strip(nc.sync, 0)
            oct_piece(nc.gpsimd, 0, HOCT)          # oct0 first half (ki 0-3)
            oct_piece(nc.scalar, HOCT, OCT)        # oct0 second half
            wsb_strip(nc.sync, 1)
            wsb_strip(nc.gpsimd, 2)
            wsb_strip(nc.scalar, 3)
            for o in range(1, NO):                 # octants 1-7 alternate rings
                eng = nc.gpsimd if o % 2 == 1 else nc.scalar
                oct_piece(eng, o * OCT, (o + 1) * OCT)
            nc.scalar.dma_start(wbc_sb[:, :], wbc[:, :])
            mk0 = mask_pool.tile([128, NC_T], F32, tag="mask")
            nc.gpsimd.dma_start(mk0[:, :], maskt[0, :, :])

            def load_batch(b, eng):
                et = enc_pool.tile([128, NO * OCT], BF16, tag="encT")
                q = NO * OCT // 4
                for i in range(4):
                    eng.dma_start(et[:, i * q : (i + 1) * q],
                                  encT[b, :, i * q : (i + 1) * q])
                mk = mask_pool.tile([128, NC_T], F32, tag="mask")
                eng.dma_start(mk[:, :], maskt[b, :, :])
                return et, mk

            # batch 1 queues behind batch 0's octants on the gpsimd ring
            prefetched = {1: load_batch(1, nc.gpsimd)}

            # ---- PE warmup during the DMA wait: junk matmuls with no deps
            # keep the HAM activity window busy so the real stream starts at
            # the full 2.4 GHz clock. ----
            junk = pset_pool.tile([128, 128], F32, tag="junk")
            for i in range(NWARM):
                nc.tensor.matmul(junk[:, :], onesb[:, :], onesb[:, :],
                                 start=(i == 0), stop=(i == NWARM - 1))

            # ---- tails (deferred into the next batch's MM stream) ----
            def emit_tail_a(b, e_t, mk):
                u = tail_pool.tile([128, NC_T], F32, tag="u")
                nc.scalar.activation(u[:, :], e_t[:, :], AF.Exp)
                wu = tail_pool.tile([128, NC_T], F32, tag="wu")
                ws = tail_pool.tile([128, 1], F32, tag="ws")
                if USE_TTR:
                    nc.vector.tensor_tensor_reduce(
                        out=wu[:, :], in0=u[:, :], in1=mk[:, :], scale=1.0,
                        scalar=0.0, op0=ALU.mult, op1=ALU.add, accum_out=ws[:, :],
                    )
                else:
                    nc.vector.tensor_mul(wu[:, :], u[:, :], mk[:, :])
                    nc.vector.reduce_sum(ws[:, :], wu[:, :], axis=mybir.AxisListType.X)
                return b, wu, ws

            def emit_tail_b(b, wu, ws):
                tot = pset_pool.tile([128, 1], F32, tag="tot")
                nc.tensor.matmul(tot[:, :], ones128[:, :], ws[:, :],
                                 start=True, stop=True)
                r = tail_pool.tile([128, 1], F32, tag="r")
                nc.vector.reciprocal(r[:, :], tot[:, :])
                al = tail_pool.tile([128, NC_T], F32, tag="al")
                nc.vector.tensor_scalar_mul(al[:, :], wu[:, :], r[:, 0:1])
                nc.sync.dma_start(out[b, :, :], al[:, :])

            # ---- main loop ----
            pending_tail = None
            tail_mid = None
            fin = {}
            for b in range(BPC):
                if b == 0:
                    et, mk = et0, mk0
                else:
                    et, mk = prefetched.pop(b)
                e_t = tail_pool.tile([128, NC_T], F32, tag="e")
                for c in range(NC_T):
                    pacc = pacc_pool.tile([128, ATTN], F32)
                    o, half = c // 2, c % 2
                    last_split = b == BPC - 1 and c == NC_T - 1 and SPLIT_FINAL
                    if last_split:
                        # final chunk as two 256-col half-groups in SEPARATE
                        # PSUM tiles (deps are tile-granular) so half 0's
                        # epilogue overlaps half 1's matmuls
                        paccb = pset_pool.tile([128, 256], F32, tag="paccb")
                        for h, pt in ((0, pacc), (1, paccb)):
                            for ki in range(KCH):
                                col0 = o * OCT + ki * 256 + half * 128
                                nc.tensor.matmul(
                                    pt[:, h * 256 : (h + 1) * 256] if h == 0 else pt[:, :],
                                    et[:, col0 : col0 + 128],
                                    wsb_sb[:, ki * ATTN + h * 256 : ki * ATTN + (h + 1) * 256],
                                    start=(ki == 0), stop=(ki == KCH - 1),
                                )
                    else:
                        for ki in range(KCH):
                            col0 = o * OCT + ki * 256 + half * 128
                            nc.tensor.matmul(
                                pacc[:, :],
                                et[:, col0 : col0 + 128],
                                wsb_sb[:, ki * ATTN : (ki + 1) * ATTN],
                                start=(ki == 0), stop=(ki == KCH - 1),
                            )
                    if pending_tail is not None and c == 2:
                        tail_mid = emit_tail_a(*pending_tail)
                        pending_tail = None
                    if tail_mid is not None and c == 6:
                        emit_tail_b(*tail_mid)
                        tail_mid = None
                    if b == 0 and c == 6:
                        prefetched[2] = load_batch(2, nc.gpsimd)
                    if b == 1 and c == 0:
                        prefetched[3] = load_batch(3, nc.gpsimd)
                    if last_split:
                        th = th_pool.tile([128, ATTN], BF16)
                        scr = scr_pool.tile([128, ATTN], BF16)
                        s01 = tail_pool.tile([128, 2], F32, tag="s01")
                        for h, pt in ((0, pacc[:, 0:256]), (1, paccb[:, :])):
                            hs = slice(h * 256, (h + 1) * 256)
                            nc.scalar.activation(th[:, hs], pt, AF.Tanh)
                            nc.vector.tensor_mul(scr[:, hs], th[:, hs], wbc_sb[:, hs])
                            nc.vector.reduce_sum(s01[:, h : h + 1], scr[:, hs],
                                                 axis=mybir.AxisListType.X)
                        nc.vector.tensor_add(e_t[:, c : c + 1], s01[:, 0:1],
                                             s01[:, 1:2])
                        continue
                    th = th_pool.tile([128, ATTN], BF16)
                    nc.scalar.activation(th[:, :], pacc[:, :], AF.Tanh)
                    scr = scr_pool.tile([128, ATTN], BF16)
                    if USE_TTR:
                        nc.vector.tensor_tensor_reduce(
                            out=scr[:, :], in0=th[:, :], in1=wbc_sb[:, :], scale=1.0,
                            scalar=0.0, op0=ALU.mult, op1=ALU.add,
                            accum_out=e_t[:, c : c + 1],
                        )
                    else:
                        nc.vector.tensor_mul(scr[:, :], th[:, :], wbc_sb[:, :])
                        nc.vector.reduce_sum(e_t[:, c : c + 1], scr[:, :],
                                             axis=mybir.AxisListType.X)
                    # final batch: overlap the bulk of the tail (cols 0..14)
                    # with chunk 15's matmuls
                    if b == BPC - 1 and SPLIT_FINAL and c == NC_T - 2:
                        u = tail_pool.tile([128, NC_T], F32, tag="u")
                        nc.scalar.activation(u[:, 0 : NC_T - 1],
                                             e_t[:, 0 : NC_T - 1], AF.Exp)
                        wu = tail_pool.tile([128, NC_T], F32, tag="wu")
                        ws = tail_pool.tile([128, 1], F32, tag="ws")
                        nc.vector.tensor_mul(wu[:, 0 : NC_T - 1],
                                             u[:, 0 : NC_T - 1],
                                             mk[:, 0 : NC_T - 1])
                        nc.vector.reduce_sum(ws[:, :], wu[:, 0 : NC_T - 1],
                                             axis=mybir.AxisListType.X)
                        fin = {"u": u, "wu": wu, "ws": ws}
                if b < BPC - 1:
                    pending_tail = (b, e_t, mk)
                elif SPLIT_FINAL:
                    # exposed part of the final tail: only column 15
                    u, wu, ws = fin["u"], fin["wu"], fin["ws"]
                    c = NC_T - 1
                    nc.scalar.activation(u[:, c : c + 1], e_t[:, c : c + 1], AF.Exp)
                    nc.vector.tensor_mul(wu[:, c : c + 1], u[:, c : c + 1],
                                         mk[:, c : c + 1])
                    ws2 = tail_pool.tile([128, 1], F32, tag="ws2")
                    nc.vector.tensor_add(ws2[:, :], ws[:, :], wu[:, c : c + 1])
                    emit_tail_b(b, wu, ws2)
                else:
                    emit_tail_b(*emit_tail_a(b, e_t, mk))

    nc.compile()
    return nc


_NC_CACHE = None


def get_nc():
    global _NC_CACHE
    if _NC_CACHE is None:
        _NC_CACHE = build_nc()
    return _NC_CACHE


def make_in_maps(enc_output, prev_dec_hidden, prev_alpha, mask,
                 W_conv, W_c2s, W_enc, b_enc, W_dec, w_score):
    enc_output = np.asarray(enc_output, np.float32)
    h = np.asarray(prev_dec_hidden, np.float32)
    pa = np.asarray(prev_alpha, np.float32)[:, 0, :]
    mask = np.asarray(mask, np.float32)
    W_enc = np.asarray(W_enc, np.float32)
    Wc = np.asarray(W_conv, np.float32).reshape(NK, KW)
    Wcs = np.asarray(W_c2s, np.float32)
    b_enc = np.asarray(b_enc, np.float32)
    W_dec = np.asarray(W_dec, np.float32)
    w_score = np.asarray(w_score, np.float32)

    # conv_out[b,t,k] = sum_j apad[b,t+j] * Wc[k,j]  (KW-wide correlation)
    apad = np.zeros((B, T + KW), np.float32)
    apad[:, PAD : PAD + T] = pa
    sw = np.lib.stride_tricks.sliding_window_view(apad, KW, axis=1)[:, :T, :]
    conv_out = np.tensordot(sw, Wc, axes=([2], [1]))          # [B,T,NK]

    # fold the rank-(NK+1) small term into enc via min-norm solve:
    # (enc + small @ W+) @ W_enc = enc @ W_enc + small
    Wp = np.linalg.pinv(W_enc)                                # [512,1024]
    D = Wcs @ Wp                                              # [NK,1024]
    bias_row = (h @ W_dec + b_enc) @ Wp                       # [B,1024]
    enchat = enc_output + np.tensordot(conv_out, D, axes=([2], [0]))
    enchat += bias_row[:, None, :]

    # octant-major pack: encT[b, p, o*OCT + ki*256 + to] =
    #   enchat[b, o*256+to, ki*128+p], t zero-padded to 2048
    ep = np.zeros((B, TPAD, ENC2), np.float32)
    ep[:, :T, :] = enchat
    encT = np.ascontiguousarray(
        ep.reshape(B, NO, 256, KCH, 128).transpose(0, 4, 1, 3, 2)
        .reshape(B, 128, NO * OCT)
    ).astype(BF)

    # W_enc packed k-chunk-major: wsb[p, ki*ATTN + a] = W_enc[ki*128 + p, a]
    wsb = np.ascontiguousarray(
        W_enc.reshape(KCH, 128, ATTN).transpose(1, 0, 2).reshape(128, KCH * ATTN)
    ).astype(BF)
    wbc = np.ascontiguousarray(
        np.broadcast_to(w_score[None, :], (128, ATTN))
    ).astype(BF)

    # mask in t-chunk-major tile layout with zero padding
    maskt = np.zeros((B, 128, NC_T), np.float32)
    mpad = np.zeros((B, TPAD), np.float32)
    mpad[:, :T] = mask
    maskt[:, :, :] = mpad.reshape(B, NC_T, 128).transpose(0, 2, 1)

    in_maps = []
    for cix in range(NCORES):
        s = slice(cix * BPC, (cix + 1) * BPC)
        in_maps.append(
            {
                "encT": np.ascontiguousarray(encT[s]),
                "wsb": wsb,
                "wbc": wbc,
                "maskt": np.ascontiguousarray(maskt[s]),
            }
        )
    return in_maps


def assemble_output(results) -> np.ndarray:
    outs = [np.asarray(results[c]["out"], np.float32) for c in range(NCORES)]
    full = np.concatenate(outs, axis=0)  # [B, 128, NC_T]
    alpha = full.transpose(0, 2, 1).reshape(B, NC_T * 128)[:, :T]
    return np.ascontiguousarray(alpha).reshape(B, 1, T)


def kernel(**inputs) -> np.ndarray:
    from concourse.bass_utils import run_bass_kernel_spmd

    nc = get_nc()
    in_maps = make_in_maps(**inputs)
    res = run_bass_kernel_spmd(nc, in_maps, core_ids=list(range(NCORES)))
    return assemble_output(res.results)


# revision 18
# speedup vs baseline: 1.0024x; 1.0024x over previous
"""Trainium2 Bass kernel for location-sensitive attention.

alpha = softmax(w_score . tanh(enc @ W_enc + b_enc + h @ W_dec + conv(prev_alpha) @ W_c2s)) * mask

Sharding: data-parallel over batch B=32 across 8 cores (4 batches/core).
All weights replicated. Full inputs in, full output out.

v3 design (vs v2): the rank-14 correction term (dec_e + b_enc + conv term)
is folded INTO the enc operand on the host via the pseudo-inverse of W_enc:
    (enc + small @ pinv(W_enc)) @ W_enc = enc @ W_enc + small
so the device runs a PURE 8-matmul-per-chunk stream -- the 9th (Mext)
matmul per chunk, the Hankel/hext DMA, and the Mext setup group are all
gone.  Min-norm spreads the correction across all 1024 contraction dims,
so bf16 error grows only ~1.2x (measured 1.03e-2 vs 2e-2 gate).

Device layout:
  - enchat is packed octant-major: encT[b, p, o*2048 + ki*256 + to] =
    enchat[b, o*256+to, ki*128+p], t zero-padded to 2048.  Each t-chunk
    (128 cols) and each DMA slice is contiguous; padded t rows produce
    e=0 which the mask kills in the softmax tail.
  - Per chunk: 8 accumulating matmuls (lhsT = enc chunk [128k, 128t],
    rhs = W chunk [128k, 512a]) -> tanh (Scalar ACT) -> fused
    tensor_tensor_reduce (x w_score, sum over a) -> e_t column.
  - DMA is issue-cost-aware (each dma_start costs ~0.7us of sequencer
    time): batch 0 lands as 8 octant slices + 4 wsb strips spread over
    the sync/scalar/gpsimd rings in consumption order; batches 1-3
    prefetch as 2 big strips each.  ~25 junk warmup matmuls run during
    the initial DMA wait to take the PE HAM throttle to 8/8 before the
    real stream starts.
  - softmax tail as v2 (exp, fused mask-mul+reduce, ones-matmul partition
    total, reciprocal, scale), deferred into the next batch's MM stream;
    the final batch's tail is split so only column 15's exp is exposed.
"""

import os
import sys
import numpy as np

for _p in ("/opt/trn_rl_repo", "/root/.axon_site/_ro/trn_rl_repo"):
    if os.path.isdir(_p) and _p not in sys.path:
        sys.path.append(_p)

import ml_dtypes
import concourse.bass as bass
import concourse.bacc as bacc
import concourse.mybir as mybir
from concourse import bass_isa
from concourse import tile

B, T, ENC2, DEC, ATTN = 32, 2000, 1024, 512, 512
NK, KW, PAD = 10, 100, 50
NCORES = 8
BPC = B // NCORES  # batches per core

F32 = mybir.dt.float32
BF16 = mybir.dt.bfloat16
AF = mybir.ActivationFunctionType
ALU = mybir.AluOpType
BF = ml_dtypes.bfloat16

KCH = ENC2 // 128   # 8 k-chunks of the 1024 contraction
NC_T = 16           # t-chunks of 128 (chunk 15 is 80 real + 48 zero pad)
TPAD = NC_T * 128   # 2048
NO = 8              # octants of 256 t
OCT = KCH * 256     # 2048 cols per octant in the packed layout
NWARM = int(os.environ.get("K_NWARM", "40"))   # junk warmup matmuls
# tensor_tensor_reduce wedges the device (NRT_EXEC_UNIT_UNRECOVERABLE) despite
# passing CoreSim -- keep the separate mul+reduce path.
USE_TTR = os.environ.get("K_TTR", "0") == "1"
SPLIT_FINAL = os.environ.get("K_SPLITFIN", "1") == "1"


def build_nc():
    nc = bacc.Bacc(None, target_bir_lowering=False)

    encT = nc.declare_dram_parameter("encT", [BPC, 128, NO * OCT], BF16, isOutput=False)
    wsb = nc.declare_dram_parameter("wsb", [128, KCH * ATTN], BF16, isOutput=False)
    wbc = nc.declare_dram_parameter("wbc", [128, ATTN], BF16, isOutput=False)
    maskt = nc.declare_dram_parameter("maskt", [BPC, 128, NC_T], F32, isOutput=False)
    out = nc.declare_dram_parameter("out", [BPC, 128, NC_T], F32, isOutput=True)

    with tile.TileContext(nc) as tc:
        with (
            tc.tile_pool(name="const", bufs=1) as cpool,
            tc.tile_pool(name="enc", bufs=3) as enc_pool,
            tc.tile_pool(name="th", bufs=3) as th_pool,
            tc.tile_pool(name="scr", bufs=2) as scr_pool,
            tc.tile_pool(name="tail", bufs=2) as tail_pool,
            tc.tile_pool(name="mask", bufs=4) as mask_pool,
            tc.tile_pool(name="pacc", bufs=5, space="PSUM") as pacc_pool,
            tc.tile_pool(name="pset", bufs=1, space="PSUM") as pset_pool,
        ):
            # ---- constants: memsets on the Vector engine so the gpsimd ring
            # can start issuing DMAs immediately ----
            onesb = cpool.tile([128, 128], BF16)
            nc.vector.memset(onesb[:, :], 1.0)
            ones128 = cpool.tile([128, 128], F32)
            nc.vector.memset(ones128[:, :], 1.0)

            wsb_sb = cpool.tile([128, KCH * ATTN], BF16)
            wbc_sb = cpool.tile([128, ATTN], BF16)

            # ---- startup DMAs spread over the three DGE rings.  Rings
            # pipeline only a few outstanding transfers and all share the 16
            # DMA queues, so queue-ENTRY order is delivery order.  The
            # chunk-0 critical set (wsb 1MB + octant 0) is the first 1-2
            # issues on every ring so it drains at full aggregate bandwidth;
            # later octants and the batch-1 prefetch queue up behind it. ----
            et0 = enc_pool.tile([128, NO * OCT], BF16, tag="encT")
            HOCT = OCT // 2

            def wsb_strip(eng, s):
                eng.dma_start(wsb_sb[:, s * 2 * ATTN : (s + 1) * 2 * ATTN],
                              wsb[:, s * 2 * ATTN : (s + 1) * 2 * ATTN])

            def oct_piece(eng, lo, hi):
                eng.dma_start(et0[:, lo:hi], encT[0, :, lo:hi])

            wsb_strip(nc.sync, 0)
            oct_piece(nc.gpsimd, 0, HOCT)          # oct0 first half (ki 0-3)
            oct_piece(nc.scalar, HOCT, OCT)        # oct0 second half
            wsb_strip(nc.sync, 1)
            wsb_strip(nc.gpsimd, 2)
            wsb_strip(nc.scalar, 3)
            for o in range(1, NO):                 # octants 1-7 alternate rings
                eng = nc.gpsimd if o % 2 == 1 else nc.scalar
                oct_piece(eng, o * OCT, (o + 1) * OCT)
            nc.scalar.dma_start(wbc_sb[:, :], wbc[:, :])
            mk0 = mask_pool.tile([128, NC_T], F32, tag="mask")
            nc.gpsimd.dma_start(mk0[:, :], maskt[0, :, :])

            def load_batch(b, eng):
                et = enc_pool.tile([128, NO * OCT], BF16, tag="encT")
                q = NO * OCT // 4
                for i in range(4):
                    eng.dma_start(et[:, i * q : (i + 1) * q],
                                  encT[b, :, i * q : (i + 1) * q])
                mk = mask_pool.tile([128, NC_T], F32, tag="mask")
                eng.dma_start(mk[:, :], maskt[b, :, :])
                return et, mk

            # batch 1 queues behind batch 0's octants on the gpsimd ring
            prefetched = {1: load_batch(1, nc.gpsimd)}

            # ---- PE warmup during the DMA wait: junk matmuls with no deps
            # keep the HAM activity window busy so the real stream starts at
            # the full 2.4 GHz clock. ----
            junk = pset_pool.tile([128, 128], F32, tag="junk")
            for i in range(NWARM):
                nc.tensor.matmul(junk[:, :], onesb[:, :], onesb[:, :],
                                 start=(i == 0), stop=(i == NWARM - 1))

            # ---- tails (deferred into the next batch's MM stream) ----
            def emit_tail_a(b, e_t, mk):
                u = tail_pool.tile([128, NC_T], F32, tag="u")
                nc.scalar.activation(u[:, :], e_t[:, :], AF.Exp)
                wu = tail_pool.tile([128, NC_T], F32, tag="wu")
                ws = tail_pool.tile([128, 1], F32, tag="ws")
                if USE_TTR:
                    nc.vector.tensor_tensor_reduce(
                        out=wu[:, :], in0=u[:, :], in1=mk[:, :], scale=1.0,
                        scalar=0.0, op0=ALU.mult, op1=ALU.add, accum_out=ws[:, :],
                    )
                else:
                    nc.vector.tensor_mul(wu[:, :], u[:, :], mk[:, :])
                    nc.vector.reduce_sum(ws[:, :], wu[:, :], axis=mybir.AxisListType.X)
                return b, wu, ws

            def emit_tail_b(b, wu, ws):
                tot = pset_pool.tile([128, 1], F32, tag="tot")
                nc.tensor.matmul(tot[:, :], ones128[:, :], ws[:, :],
                                 start=True, stop=True)
                r = tail_pool.tile([128, 1], F32, tag="r")
                nc.vector.reciprocal(r[:, :], tot[:, :])
                al = tail_pool.tile([128, NC_T], F32, tag="al")
                nc.vector.tensor_scalar_mul(al[:, :], wu[:, :], r[:, 0:1])
                nc.sync.dma_start(out[b, :, :], al[:, :])

            # ---- main loop ----
            pending_tail = None
            tail_mid = None
            fin = {}
            for b in range(BPC):
                if b == 0:
                    et, mk = et0, mk0
                else:
                    et, mk = prefetched.pop(b)
                e_t = tail_pool.tile([128, NC_T], F32, tag="e")
                for c in range(NC_T):
                    pacc = pacc_pool.tile([128, ATTN], F32)
                    o, half = c // 2, c % 2
                    last_split = b == BPC - 1 and c == NC_T - 1 and SPLIT_FINAL
                    if last_split:
                        # final chunk as two 256-col half-groups in SEPARATE
                        # PSUM tiles (deps are tile-granular) so half 0's
                        # epilogue overlaps half 1's matmuls
                        paccb = pset_pool.tile([128, 256], F32, tag="paccb")
                        for h, pt in ((0, pacc), (1, paccb)):
                            for ki in range(KCH):
                                col0 = o * OCT + ki * 256 + half * 128
                                nc.tensor.matmul(
                                    pt[:, h * 256 : (h + 1) * 256] if h == 0 else pt[:, :],
                                    et[:, col0 : col0 + 128],
                                    wsb_sb[:, ki * ATTN + h * 256 : ki * ATTN + (h + 1) * 256],
                                    start=(ki == 0), stop=(ki == KCH - 1),
                                )
                    else:
                        for ki in range(KCH):
                            col0 = o * OCT + ki * 256 + half * 128
                            nc.tensor.matmul(
                                pacc[:, :],
                                et[:, col0 : col0 + 128],
                                wsb_sb[:, ki * ATTN : (ki + 1) * ATTN],
                                start=(ki == 0), stop=(ki == KCH - 1),
                            )
                    if pending_tail is not None and c == 2:
                        tail_mid = emit_tail_a(*pending_tail)
                        pending_tail = None
                    if tail_mid is not None and c == 6:
                        emit_tail_b(*tail_mid)
                        tail_mid = None
                    if b == 0 and c == 6:
                        prefetched[2] = load_batch(2, nc.gpsimd)
                    if b == 1 and c == 0:
                        prefetched[3] = load_batch(3, nc.gpsimd)
                    if last_split:
                        th = th_pool.tile([128, ATTN], BF16)
                        scr = scr_pool.tile([128, ATTN], BF16)
                        s01 = tail_pool.tile([128, 2], F32, tag="s01")
                        for h, pt in ((0, pacc[:, 0:256]), (1, paccb[:, :])):
                            hs = slice(h * 256, (h + 1) * 256)
                            nc.scalar.activation(th[:, hs], pt, AF.Tanh)
                            nc.vector.tensor_mul(scr[:, hs], th[:, hs], wbc_sb[:, hs])
                            nc.vector.reduce_sum(s01[:, h : h + 1], scr[:, hs],
                                                 axis=mybir.AxisListType.X)
                        nc.vector.tensor_add(e_t[:, c : c + 1], s01[:, 0:1],
                                             s01[:, 1:2])
                        continue
                    th = th_pool.tile([128, ATTN], BF16)
                    nc.scalar.activation(th[:, :], pacc[:, :], AF.Tanh)
                    scr = scr_pool.tile([128, ATTN], BF16)
                    if USE_TTR:
                        nc.vector.tensor_tensor_reduce(
                            out=scr[:, :], in0=th[:, :], in1=wbc_sb[:, :], scale=1.0,
                            scalar=0.0, op0=ALU.mult, op1=ALU.add,
                            accum_out=e_t[:, c : c + 1],
                        )
                    else:
                        nc.vector.tensor_mul(scr[:, :], th[:, :], wbc_sb[:, :])
                        nc.vector.reduce_sum(e_t[:, c : c + 1], scr[:, :],
                                             axis=mybir.AxisListType.X)
                    # final batch: overlap the bulk of the tail (cols 0..14)
                    # with chunk 15's matmuls
                    if b == BPC - 1 and SPLIT_FINAL and c == NC_T - 2:
                        u = tail_pool.tile([128, NC_T], F32, tag="u")
                        nc.scalar.activation(u[:, 0 : NC_T - 1],
                                             e_t[:, 0 : NC_T - 1], AF.Exp)
                        wu = tail_pool.tile([128, NC_T], F32, tag="wu")
                        ws = tail_pool.tile([128, 1], F32, tag="ws")
                        nc.vector.tensor_mul(wu[:, 0 : NC_T - 1],
                                             u[:, 0 : NC_T - 1],
                                             mk[:, 0 : NC_T - 1])
                        nc.vector.reduce_sum(ws[:, :], wu[:, 0 : NC_T - 1],
                                             axis=mybir.AxisListType.X)
                        fin = {"u": u, "wu": wu, "ws": ws}
                if b < BPC - 1:
                    pending_tail = (b, e_t, mk)
                elif SPLIT_FINAL:
                    # exposed part of the final tail: only column 15
                    u, wu, ws = fin["u"], fin["wu"], fin["ws"]
                    c = NC_T - 1
                    nc.scalar.activation(u[:, c : c + 1], e_t[:, c : c + 1], AF.Exp)
                    nc.vector.tensor_mul(wu[:, c : c + 1], u[:, c : c + 1],
                                         mk[:, c : c + 1])
                    ws2 = tail_pool.tile([128, 1], F32, tag="ws2")
                    nc.vector.tensor_add(ws2[:, :], ws[:, :], wu[:, c : c + 1])
                    emit_tail_b(b, wu, ws2)
                else:
                    emit_tail_b(*emit_tail_a(b, e_t, mk))

    nc.compile()
    return nc


_NC_CACHE = None


def get_nc():
    global _NC_CACHE
    if _NC_CACHE is None:
        _NC_CACHE = build_nc()
    return _NC_CACHE


def make_in_maps(enc_output, prev_dec_hidden, prev_alpha, mask,
                 W_conv, W_c2s, W_enc, b_enc, W_dec, w_score):
    enc_output = np.asarray(enc_output, np.float32)
    h = np.asarray(prev_dec_hidden, np.float32)
    pa = np.asarray(prev_alpha, np.float32)[:, 0, :]
    mask = np.asarray(mask, np.float32)
    W_enc = np.asarray(W_enc, np.float32)
    Wc = np.asarray(W_conv, np.float32).reshape(NK, KW)
    Wcs = np.asarray(W_c2s, np.float32)
    b_enc = np.asarray(b_enc, np.float32)
    W_dec = np.asarray(W_dec, np.float32)
    w_score = np.asarray(w_score, np.float32)

    # conv_out[b,t,k] = sum_j apad[b,t+j] * Wc[k,j]  (KW-wide correlation)
    apad = np.zeros((B, T + KW), np.float32)
    apad[:, PAD : PAD + T] = pa
    sw = np.lib.stride_tricks.sliding_window_view(apad, KW, axis=1)[:, :T, :]
    conv_out = np.tensordot(sw, Wc, axes=([2], [1]))          # [B,T,NK]

    # fold the rank-(NK+1) small term into enc via min-norm solve:
    # (enc + small @ W+) @ W_enc = enc @ W_enc + small
    Wp = np.linalg.pinv(W_enc)                                # [512,1024]
    D = Wcs @ Wp                                              # [NK,1024]
    bias_row = (h @ W_dec + b_enc) @ Wp                       # [B,1024]
    enchat = enc_output + np.tensordot(conv_out, D, axes=([2], [0]))
    enchat += bias_row[:, None, :]

    # octant-major pack: encT[b, p, o*OCT + ki*256 + to] =
    #   enchat[b, o*256+to, ki*128+p], t zero-padded to 2048
    ep = np.zeros((B, TPAD, ENC2), np.float32)
    ep[:, :T, :] = enchat
    encT = np.ascontiguousarray(
        ep.reshape(B, NO, 256, KCH, 128).transpose(0, 4, 1, 3, 2)
        .reshape(B, 128, NO * OCT)
    ).astype(BF)

    # W_enc packed k-chunk-major: wsb[p, ki*ATTN + a] = W_enc[ki*128 + p, a]
    wsb = np.ascontiguousarray(
        W_enc.reshape(KCH, 128, ATTN).transpose(1, 0, 2).reshape(128, KCH * ATTN)
    ).astype(BF)
    wbc = np.ascontiguousarray(
        np.broadcast_to(w_score[None, :], (128, ATTN))
    ).astype(BF)

    # mask in t-chunk-major tile layout with zero padding
    maskt = np.zeros((B, 128, NC_T), np.float32)
    mpad = np.zeros((B, TPAD), np.float32)
    mpad[:, :T] = mask
    maskt[:, :, :] = mpad.reshape(B, NC_T, 128).transpose(0, 2, 1)

    in_maps = []
    for cix in range(NCORES):
        s = slice(cix * BPC, (cix + 1) * BPC)
        in_maps.append(
            {
                "encT": np.ascontiguousarray(encT[s]),
                "wsb": wsb,
                "wbc": wbc,
                "maskt": np.ascontiguousarray(maskt[s]),
            }
        )
    return in_maps


def assemble_output(results) -> np.ndarray:
    outs = [np.asarray(results[c]["out"], np.float32) for c in range(NCORES)]
    full = np.concatenate(outs, axis=0)  # [B, 128, NC_T]
    alpha = full.transpose(0, 2, 1).reshape(B, NC_T * 128)[:, :T]
    return np.ascontiguousarray(alpha).reshape(B, 1, T)


def kernel(**inputs) -> np.ndarray:
    from concourse.bass_utils import run_bass_kernel_spmd

    nc = get_nc()
    in_maps = make_in_maps(**inputs)
    res = run_bass_kernel_spmd(nc, in_maps, core_ids=list(range(NCORES)))
    return assemble_output(res.results)
